# revision 16
# baseline (speedup 1.0000x reference)
"""PSLoRA linear layer on 8 Trainium2 NeuronCores (Bass/Tile, fp8 DoubleRow).

out[b] = x[b] @ W.T + bias + 0.5 * (x[b] @ lora_A[idx[b]]) @ lora_B.T

Sharding: data-parallel over batch (B=8 -> one batch element per core).
W / lora params are replicated; the per-core lora_A gather happens on host
(index has only 8 entries).

The main GEMM runs entirely in fp8-e4m3 DoubleRow (0.5 cycles/row,
HW-measured ~127 ns per matmul covering TWO k-tiles vs ~250 ns for one
bf16 k-tile) using a 3-pass residual decomposition that cancels
quantization error to second order:

    x = xh + xl,  64W = Wh + Wl  (each term fp8(residual))
    x @ W.T ~= (xh@Wh.T + xh@Wl.T + xl@Wh.T) / 64     [xl@Wl dropped]

Host-verified rel err 3.0e-3 vs the 2e-2 gate — same accuracy as pure
bf16. All W-side operands carry a x64 scale so fp8 stays in its normal
range; every PSUM accumulation is 64x and evictions scale by 1/64.
The LoRA delta and bias fold into the same accumulation group via one
bf16 K=33 matmul (32 axT rows + ones row against 64*[0.5*B^T; bias]).

Output is computed transposed ([DOUT,S] tiles, W-block stationary) and
written back bf16 (host transposes and casts to f32). o-blocks alternate
PSUM bank sets {0-3}/{4-7} so evictions overlap the next block. DMA
rings split by traffic class: x on gpsimd, W on sync, stores on scalar.
"""
import sys
sys.path.insert(0, "/opt/trn_rl_repo")
import numpy as np

B, S, DIN, DOUT, R = 8, 2048, 4096, 4096, 32
LORA_SCALING = 16 / 32
KT = DIN // 128          # 32 contraction tiles
NP8 = KT // 2            # 16 DoubleRow k-pairs
OB2 = DOUT // 128        # 32 output o-blocks
XC8 = 4                  # x chunks per pass (4 k-pairs each)
WSCALE = 64.0
N_CORES = 8

_cache = {}


def _build(hw_loop=1):
    import concourse.bacc as bacc
    import concourse.mybir as mybir
    from concourse.tile import TileContext

    BF16 = mybir.dt.bfloat16
    FP8 = mybir.dt.float8e4
    F32 = mybir.dt.float32
    DR = mybir.MatmulPerfMode.DoubleRow

    nc = bacc.Bacc()
    # [p, t, a, s] = fp8 x / residual: x[s, (2t+a)*128+p]
    x8h = nc.dram_tensor("x8h", [128, NP8 * 2 * S], FP8, kind="ExternalInput")
    x8l = nc.dram_tensor("x8l", [128, NP8 * 2 * S], FP8, kind="ExternalInput")
    # [p, ob, t, a, m] = fp8 of 64*W / residual: W[ob*128+m, (2t+a)*128+p]
    W8h = nc.dram_tensor("W8h", [128, OB2 * NP8 * 2 * 128], FP8,
                         kind="ExternalInput")
    W8l = nc.dram_tensor("W8l", [128, OB2 * NP8 * 2 * 128], FP8,
                         kind="ExternalInput")
    # [p, t, a, r] = fp8(64*A)
    A8R = nc.dram_tensor("A8R", [128, NP8 * 2 * R], FP8, kind="ExternalInput")
    # rows 0-31: 64*0.5*lora_B.T, row 32: 64*bias  (bf16)
    BTa = nc.dram_tensor("BTa", [R + 1, DOUT], BF16, kind="ExternalInput")
    ONES = nc.dram_tensor("ONES", [1, 512], BF16, kind="ExternalInput")
    # [p, ob, s]: outT[ob*128+p, s]
    out = nc.dram_tensor("out", [128, OB2, S], BF16, kind="ExternalOutput")

    with TileContext(nc) as tc:
        with (
            tc.tile_pool(name="xp", bufs=XC8) as xp,
            tc.tile_pool(name="wp", bufs=3) as wp,
            tc.tile_pool(name="cp", bufs=1) as cp,
            tc.tile_pool(name="axp", bufs=4) as axp,
            tc.tile_pool(name="op", bufs=2) as op_,
            tc.tile_pool(name="pp", bufs=1, space="PSUM") as pp,
        ):
            a8 = cp.tile([128, NP8, 2, R], FP8, name="a8")
            nc.sync.dma_start(
                a8, A8R[:, :].rearrange("p (t a r) -> p t a r", t=NP8, a=2))
            bt = cp.tile([R + 1, DOUT], BF16, name="bt")
            nc.sync.dma_start(bt, BTa[:, :])

            def body():
                xh, xl = [], []
                for src, dst in ((x8h, xh), (x8l, xl)):
                    for j in range(XC8):
                        t = xp.tile([128, NP8 // XC8, 2, S], FP8,
                                    name="xh" if dst is xh else "xl")
                        lo = j * (NP8 // XC8) * 2 * S
                        nc.gpsimd.dma_start(
                            t, src[:, lo:lo + (NP8 // XC8) * 2 * S].rearrange(
                                "p (t a s) -> p t a s", t=NP8 // XC8, a=2))
                        dst.append(t)

                def xsl(xlist, t, c):
                    return xlist[t // XC8][:, t % XC8, :,
                                           c * 512:(c + 1) * 512]

                # axT (64*ax in psum; evict scales 1/64) + ones row
                axc = []
                for c in range(S // 512):
                    pa = pp.tile([R, 512], F32, name=f"ps{c}")
                    for t in range(NP8):
                        nc.tensor.matmul(
                            pa, lhsT=a8[:, t, :, :], rhs=xsl(xh, t, c),
                            start=(t == 0), stop=(t == NP8 - 1),
                            perf_mode=DR)
                    axt = axp.tile([R + 1, 512], BF16, name="axt")
                    nc.scalar.mul(axt[0:R, :], pa, 1.0 / WSCALE)
                    nc.scalar.dma_start(axt[R:R + 1, :], ONES[0:1, :])
                    axc.append(axt)
                # main: per o-block pair; W-block stationary across the 4
                # S-chunks; 3 residual passes per k-pair; banks {0-3}/{4-7}
                # alternate per ob so evictions overlap.
                for j in range(OB2 // 2):
                    sz = NP8 * 2 * 128
                    wh = wp.tile([128, 2, NP8, 2, 128], FP8, name="wh")
                    nc.sync.dma_start(
                        wh, W8h[:, 2 * j * sz:(2 * j + 2) * sz].rearrange(
                            "p (o t a m) -> p o t a m", o=2, t=NP8, a=2))
                    wl = wp.tile([128, 2, NP8, 2, 128], FP8, name="wl")
                    nc.sync.dma_start(
                        wl, W8l[:, 2 * j * sz:(2 * j + 2) * sz].rearrange(
                            "p (o t a m) -> p o t a m", o=2, t=NP8, a=2))
                    for par in range(2):
                        ob = 2 * j + par
                        ps = [pp.tile([128, 512], F32, name=f"ps{par * 4 + c}")
                              for c in range(4)]
                        for t in range(NP8):
                            whs = wh[:, par, t, :, :]
                            wls = wl[:, par, t, :, :]
                            for c in range(4):
                                nc.tensor.matmul(
                                    ps[c], lhsT=whs, rhs=xsl(xh, t, c),
                                    start=(t == 0), stop=False, perf_mode=DR)
                            for c in range(4):
                                nc.tensor.matmul(
                                    ps[c], lhsT=whs, rhs=xsl(xl, t, c),
                                    start=False, stop=False, perf_mode=DR)
                            for c in range(4):
                                nc.tensor.matmul(
                                    ps[c], lhsT=wls, rhs=xsl(xh, t, c),
                                    start=False, stop=False, perf_mode=DR)
                        btsl = bt[:, ob * 128:(ob + 1) * 128]
                        for c in range(4):
                            nc.tensor.matmul(
                                ps[c], lhsT=btsl, rhs=axc[c][:, :],
                                start=False, stop=True)
                        if par == 0:
                            st = op_.tile([128, 2, 4 * 512], BF16, name="st")
                        for c in range(4):
                            dst = st[:, par, c * 512:(c + 1) * 512]
                            if c % 2 == 0:
                                nc.vector.tensor_scalar_mul(
                                    dst, ps[c], 1.0 / WSCALE)
                            else:
                                nc.scalar.mul(dst, ps[c], 1.0 / WSCALE)
                        if par == 1:
                            nc.scalar.dma_start(
                                out[:, 2 * j:2 * j + 2, :], st[:, :, :])

            if hw_loop > 1:
                with tc.For_i(0, hw_loop, 1):
                    body()
            else:
                body()
    nc.finalize()
    return nc


def _prep_in_maps(input, weight, bias, lora_A, lora_B, labeler_index):
    import ml_dtypes
    bf16 = ml_dtypes.bfloat16
    fp8 = ml_dtypes.float8_e4m3fn

    x = np.asarray(input, dtype=np.float32)
    W = np.asarray(weight, dtype=np.float32)
    bias = np.asarray(bias, dtype=np.float32)
    lA = np.asarray(lora_A, dtype=np.float32)
    lB = np.asarray(lora_B, dtype=np.float32)
    idx = np.asarray(labeler_index).astype(np.int64)

    def wlay(a):  # [DOUT, DIN] -> [p, ob, t, a, m]
        return np.ascontiguousarray(
            a.reshape(OB2, 128, NP8, 2, 128).transpose(4, 0, 2, 3, 1)
        ).reshape(128, OB2 * NP8 * 2 * 128)

    W64 = WSCALE * W
    Wh8 = W64.astype(fp8)
    Wl8 = (W64 - Wh8.astype(np.float32)).astype(fp8)
    W8hr = wlay(Wh8.astype(np.float32)).astype(fp8)
    W8lr = wlay(Wl8.astype(np.float32)).astype(fp8)
    BTa = np.concatenate(
        [WSCALE * LORA_SCALING * lB.T, WSCALE * bias[None, :]],
        axis=0).astype(bf16)
    ones = np.ones((1, 512), dtype=bf16)

    def xlay(a):  # [S, DIN] -> [p, t, a, s]
        return np.ascontiguousarray(
            a.reshape(S, NP8, 2, 128).transpose(3, 1, 2, 0)
        ).reshape(128, NP8 * 2 * S)

    in_maps = []
    for b in range(B):
        xhi = x[b].astype(fp8)
        xlo = (x[b] - xhi.astype(np.float32)).astype(fp8)
        x8h = xlay(xhi.astype(np.float32)).astype(fp8)
        x8l = xlay(xlo.astype(np.float32)).astype(fp8)
        A8R = np.ascontiguousarray(
            (WSCALE * lA[idx[b]]).reshape(NP8, 2, 128, R)
            .transpose(2, 0, 1, 3)
        ).astype(fp8).reshape(128, NP8 * 2 * R)
        in_maps.append({"x8h": x8h, "x8l": x8l, "W8h": W8hr, "W8l": W8lr,
                        "A8R": A8R, "BTa": BTa, "ONES": ones})
    return in_maps


def _assemble(raw):
    # raw: [128, OB2, S] bf16, [p, ob, s] = out[s, ob*128+p] -> [S, DOUT] f32
    return np.ascontiguousarray(raw.transpose(2, 1, 0)).reshape(
        S, DOUT).astype(np.float32)


def kernel(input, weight, bias, lora_A, lora_B, labeler_index):
    from concourse import bass_utils

    in_maps = _prep_in_maps(input, weight, bias, lora_A, lora_B, labeler_index)
    if "nc" not in _cache:
        _cache["nc"] = _build()
    last_err = None
    for attempt in range(3):
        try:
            res = bass_utils.run_bass_kernel_spmd(
                _cache["nc"], in_maps, core_ids=list(range(N_CORES)))
            return np.stack([_assemble(res.results[b]["out"])
                             for b in range(B)])
        except Exception as e:  # transient NRT wedge from a prior crashed run
            last_err = e
            if "UNRECOVERABLE" not in str(e) and "UNAVAILABLE" not in str(e):
                raise
    raise last_err


# revision 21
# speedup vs baseline: 1.0086x; 1.0086x over previous
"""PSLoRA linear layer on 8 Trainium2 NeuronCores (Bass/Tile, fp8 DoubleRow).

out[b] = x[b] @ W.T + bias + 0.5 * (x[b] @ lora_A[idx[b]]) @ lora_B.T

Sharding: data-parallel over batch (B=8 -> one batch element per core).
W / lora params are replicated; the per-core lora_A gather happens on host
(index has only 8 entries).

The main GEMM runs entirely in fp8-e4m3 DoubleRow (0.5 cycles/row,
HW-measured ~127 ns per matmul covering TWO k-tiles vs ~250 ns for one
bf16 k-tile) using a 3-pass residual decomposition that cancels
quantization error to second order:

    x = xh + xl,  64W = Wh + Wl  (each term fp8(residual))
    x @ W.T ~= (xh@Wh.T + xh@Wl.T + xl@Wh.T) / 64     [xl@Wl dropped]

Host-verified rel err 3.0e-3 vs the 2e-2 gate — same accuracy as pure
bf16. All W-side operands carry a x64 scale so fp8 stays in its normal
range; every PSUM accumulation is 64x and evictions scale by 1/64.
The LoRA delta and bias fold into the same accumulation group via one
bf16 K=33 matmul (32 axT rows + ones row against 64*[0.5*B^T; bias]).

Output is computed transposed ([DOUT,S] tiles, W-block stationary) and
written back bf16 (host transposes and casts to f32). o-blocks alternate
PSUM bank sets {0-3}/{4-7} so evictions overlap the next block. DMA
rings split by traffic class: x on gpsimd, W on sync, stores on scalar.
"""
import sys
sys.path.insert(0, "/opt/trn_rl_repo")
import numpy as np

B, S, DIN, DOUT, R = 8, 2048, 4096, 4096, 32
LORA_SCALING = 16 / 32
KT = DIN // 128          # 32 contraction tiles
NP8 = KT // 2            # 16 DoubleRow k-pairs
OB2 = DOUT // 128        # 32 output o-blocks
XC8 = 4                  # x chunks per pass (4 k-pairs each)
WSCALE = 64.0
N_CORES = 8

_cache = {}


def _build(hw_loop=1):
    import concourse.bacc as bacc
    import concourse.mybir as mybir
    from concourse.tile import TileContext

    BF16 = mybir.dt.bfloat16
    FP8 = mybir.dt.float8e4
    F32 = mybir.dt.float32
    DR = mybir.MatmulPerfMode.DoubleRow

    nc = bacc.Bacc()
    # [p, t, c, a, s] = fp8 x / residual: x[c*512+s, (2t+a)*128+p]
    # (the DR interleave axis `a` sits at stride 512 — matches the fast
    # HW-measured ifmap packing; stride 2048 ran at half rate)
    x8h = nc.dram_tensor("x8h", [128, NP8 * 2 * S], FP8, kind="ExternalInput")
    x8l = nc.dram_tensor("x8l", [128, NP8 * 2 * S], FP8, kind="ExternalInput")
    # [p, ob, t, a, m] = fp8 of 64*W / residual: W[ob*128+m, (2t+a)*128+p]
    W8h = nc.dram_tensor("W8h", [128, OB2 * NP8 * 2 * 128], FP8,
                         kind="ExternalInput")
    W8l = nc.dram_tensor("W8l", [128, OB2 * NP8 * 2 * 128], FP8,
                         kind="ExternalInput")
    # [p, t, a, r] = fp8(64*A)
    A8R = nc.dram_tensor("A8R", [128, NP8 * 2 * R], FP8, kind="ExternalInput")
    # rows 0-31: 64*0.5*lora_B.T, row 32: 64*bias  (bf16)
    BTa = nc.dram_tensor("BTa", [R + 1, DOUT], BF16, kind="ExternalInput")
    ONES = nc.dram_tensor("ONES", [1, 512], BF16, kind="ExternalInput")
    # [p, ob, s]: outT[ob*128+p, s]
    out = nc.dram_tensor("out", [128, OB2, S], BF16, kind="ExternalOutput")

    with TileContext(nc) as tc:
        with (
            tc.tile_pool(name="xp", bufs=XC8) as xp,
            tc.tile_pool(name="wp", bufs=3) as wp,
            tc.tile_pool(name="cp", bufs=1) as cp,
            tc.tile_pool(name="axp", bufs=4) as axp,
            tc.tile_pool(name="op", bufs=2) as op_,
            tc.tile_pool(name="pp", bufs=1, space="PSUM") as pp,
        ):
            a8 = cp.tile([128, NP8, 2, R], FP8, name="a8")
            nc.sync.dma_start(
                a8, A8R[:, :].rearrange("p (t a r) -> p t a r", t=NP8, a=2))
            bt = cp.tile([R + 1, DOUT], BF16, name="bt")
            nc.sync.dma_start(bt, BTa[:, :])

            def body():
                xh, xl = [], []
                for src, dst in ((x8h, xh), (x8l, xl)):
                    for j in range(XC8):
                        t = xp.tile([128, NP8 // XC8, S // 512, 2, 512], FP8,
                                    name="xh" if dst is xh else "xl")
                        lo = j * (NP8 // XC8) * 2 * S
                        nc.gpsimd.dma_start(
                            t, src[:, lo:lo + (NP8 // XC8) * 2 * S].rearrange(
                                "p (t c a s) -> p t c a s",
                                t=NP8 // XC8, c=S // 512, a=2))
                        dst.append(t)

                def xsl(xlist, t, c):
                    return xlist[t // (NP8 // XC8)][:, t % (NP8 // XC8),
                                                    c, :, :]

                # axT (64*ax in psum; evict scales 1/64) + ones row
                axc = []
                for c in range(S // 512):
                    pa = pp.tile([R, 512], F32, name=f"ps{c}")
                    for t in range(NP8):
                        nc.tensor.matmul(
                            pa, lhsT=a8[:, t, :, :], rhs=xsl(xh, t, c),
                            start=(t == 0), stop=(t == NP8 - 1),
                            perf_mode=DR)
                    axt = axp.tile([R + 1, 512], BF16, name="axt")
                    nc.scalar.mul(axt[0:R, :], pa, 1.0 / WSCALE)
                    nc.scalar.dma_start(axt[R:R + 1, :], ONES[0:1, :])
                    axc.append(axt)
                # main: per o-block pair; W-block stationary across the 4
                # S-chunks; 3 residual passes per k-pair; banks {0-3}/{4-7}
                # alternate per ob so evictions overlap.
                for j in range(OB2 // 2):
                    sz = NP8 * 2 * 128
                    wh = wp.tile([128, 2, NP8, 2, 128], FP8, name="wh")
                    nc.sync.dma_start(
                        wh, W8h[:, 2 * j * sz:(2 * j + 2) * sz].rearrange(
                            "p (o t a m) -> p o t a m", o=2, t=NP8, a=2))
                    wl = wp.tile([128, 2, NP8, 2, 128], FP8, name="wl")
                    nc.sync.dma_start(
                        wl, W8l[:, 2 * j * sz:(2 * j + 2) * sz].rearrange(
                            "p (o t a m) -> p o t a m", o=2, t=NP8, a=2))
                    for par in range(2):
                        ob = 2 * j + par
                        ps = [pp.tile([128, 512], F32, name=f"ps{par * 4 + c}")
                              for c in range(4)]
                        for t in range(NP8):
                            whs = wh[:, par, t, :, :]
                            wls = wl[:, par, t, :, :]
                            for c in range(4):
                                nc.tensor.matmul(
                                    ps[c], lhsT=whs, rhs=xsl(xh, t, c),
                                    start=(t == 0), stop=False, perf_mode=DR)
                            for c in range(4):
                                nc.tensor.matmul(
                                    ps[c], lhsT=whs, rhs=xsl(xl, t, c),
                                    start=False, stop=False, perf_mode=DR)
                            for c in range(4):
                                nc.tensor.matmul(
                                    ps[c], lhsT=wls, rhs=xsl(xh, t, c),
                                    start=False, stop=False, perf_mode=DR)
                        btsl = bt[:, ob * 128:(ob + 1) * 128]
                        for c in range(4):
                            nc.tensor.matmul(
                                ps[c], lhsT=btsl, rhs=axc[c][:, :],
                                start=False, stop=True)
                        if par == 0:
                            st = op_.tile([128, 2, 4 * 512], BF16, name="st")
                        for c in range(4):
                            dst = st[:, par, c * 512:(c + 1) * 512]
                            if c % 2 == 0:
                                nc.vector.tensor_scalar_mul(
                                    dst, ps[c], 1.0 / WSCALE)
                            else:
                                nc.scalar.mul(dst, ps[c], 1.0 / WSCALE)
                        if par == 1:
                            nc.scalar.dma_start(
                                out[:, 2 * j:2 * j + 2, :], st[:, :, :])

            if hw_loop > 1:
                with tc.For_i(0, hw_loop, 1):
                    body()
            else:
                body()
    nc.finalize()
    return nc


def _prep_in_maps(input, weight, bias, lora_A, lora_B, labeler_index):
    import ml_dtypes
    bf16 = ml_dtypes.bfloat16
    fp8 = ml_dtypes.float8_e4m3fn

    x = np.asarray(input, dtype=np.float32)
    W = np.asarray(weight, dtype=np.float32)
    bias = np.asarray(bias, dtype=np.float32)
    lA = np.asarray(lora_A, dtype=np.float32)
    lB = np.asarray(lora_B, dtype=np.float32)
    idx = np.asarray(labeler_index).astype(np.int64)

    def wlay(a):  # [DOUT, DIN] -> [p, ob, t, a, m]
        return np.ascontiguousarray(
            a.reshape(OB2, 128, NP8, 2, 128).transpose(4, 0, 2, 3, 1)
        ).reshape(128, OB2 * NP8 * 2 * 128)

    W64 = WSCALE * W
    Wh8 = W64.astype(fp8)
    Wl8 = (W64 - Wh8.astype(np.float32)).astype(fp8)
    W8hr = wlay(Wh8.astype(np.float32)).astype(fp8)
    W8lr = wlay(Wl8.astype(np.float32)).astype(fp8)
    BTa = np.concatenate(
        [WSCALE * LORA_SCALING * lB.T, WSCALE * bias[None, :]],
        axis=0).astype(bf16)
    ones = np.ones((1, 512), dtype=bf16)

    def xlay(a):  # [S, DIN] -> [p, t, c, a, s512]
        return np.ascontiguousarray(
            a.reshape(S // 512, 512, NP8, 2, 128).transpose(4, 2, 0, 3, 1)
        ).reshape(128, NP8 * 2 * S)

    in_maps = []
    for b in range(B):
        xhi = x[b].astype(fp8)
        xlo = (x[b] - xhi.astype(np.float32)).astype(fp8)
        x8h = xlay(xhi.astype(np.float32)).astype(fp8)
        x8l = xlay(xlo.astype(np.float32)).astype(fp8)
        A8R = np.ascontiguousarray(
            (WSCALE * lA[idx[b]]).reshape(NP8, 2, 128, R)
            .transpose(2, 0, 1, 3)
        ).astype(fp8).reshape(128, NP8 * 2 * R)
        in_maps.append({"x8h": x8h, "x8l": x8l, "W8h": W8hr, "W8l": W8lr,
                        "A8R": A8R, "BTa": BTa, "ONES": ones})
    return in_maps


def _assemble(raw):
    # raw: [128, OB2, S] bf16, [p, ob, s] = out[s, ob*128+p] -> [S, DOUT] f32
    return np.ascontiguousarray(raw.transpose(2, 1, 0)).reshape(
        S, DOUT).astype(np.float32)


def kernel(input, weight, bias, lora_A, lora_B, labeler_index):
    from concourse import bass_utils

    in_maps = _prep_in_maps(input, weight, bias, lora_A, lora_B, labeler_index)
    if "nc" not in _cache:
        _cache["nc"] = _build()
    last_err = None
    for attempt in range(3):
        try:
            res = bass_utils.run_bass_kernel_spmd(
                _cache["nc"], in_maps, core_ids=list(range(N_CORES)))
            return np.stack([_assemble(res.results[b]["out"])
                             for b in range(B)])
        except Exception as e:  # transient NRT wedge from a prior crashed run
            last_err = e
            if "UNRECOVERABLE" not in str(e) and "UNAVAILABLE" not in str(e):
                raise
    raise last_err
